# revision 8
# baseline (speedup 1.0000x reference)
"""CapsuleLayer dynamic-routing kernel for 8x Trainium2 (Bass/Tile).

Shapes (fixed): x (32, 32, 2048) f32, weight (2048, 64, 32, 32) f32.
B=32 batch, I=32 in_unit, C=2048 in_channel, U=64 num_unit, O=32 unit_size.

Strategy: shard C over the 8 cores (CL=256 channels each). Per core the
GEMM u_hat[b,c,u,o] = sum_i W[c,u,o,i] x[b,i,c] is computed as 64 groups
of 4 channels: K = (c4,i) = 128 with a block-diagonal x operand as the
PE-stationary weight, streaming W. u_hat tiles [(c4,b) | (o,u)] are kept
bf16, partly resident in SBUF and partly stored to DRAM. Routing
iterations 2 and 3 sweep the tiles once each, fusing agree_{k-1}
(mult+reduce, PE delta-matmul over b), the per-channel softmax (all u
local), and the weighted s_k accumulation (DVE mult, PE delta-matmul
into PSUM). s_k is AllReduced across cores (sum over the C shards);
squash runs on every core; core 0's v is the output.

Free-dim layout is (o, u) [u innermost] so the ck broadcast multiply
keeps the DVE 2x packed mode (innermost stride-1 for all operands).
"""

import os
import sys
import types

import numpy as np

_TRN_REPO = "/opt/trn_rl_repo"
if _TRN_REPO not in sys.path:
    sys.path.insert(0, _TRN_REPO)

import concourse.bacc as bacc  # noqa: E402
import concourse.bass as bass  # noqa: E402
import concourse.mybir as mybir  # noqa: E402
import concourse.tile as tile  # noqa: E402

NCORES = 8
B, I, C, U, O = 32, 32, 2048, 64, 32
CL = C // NCORES          # channels per core
G = CL // 4               # 64 groups of 4 channels
P = 128
FREE = U * O              # 2048, laid out (o, u): f = o*U + u
RES = int(os.environ.get("CAPS_RES", "24"))
PROD_GPSIMD = os.environ.get("CAPS_PROD_GPSIMD", "0") == "1"
EPS = 1e-8

F32 = mybir.dt.float32
BF16 = mybir.dt.bfloat16
NPBF16 = mybir.dt.np(BF16)

# interleave resident/streamed groups so stream-in DMA overlaps resident compute
_ORDER = []
_res_i = 0
_str_i = 0
for _g in range(G):
    if _g % 2 == 0 and _res_i < RES:
        _ORDER.append(("res", _res_i))
        _res_i += 1
    else:
        _ORDER.append(("str", _str_i))
        _str_i += 1
NSTR = _str_i


def build_nc():
    nc = bacc.Bacc("TRN2", target_bir_lowering=False, debug=True)

    wg = nc.declare_dram_parameter("wg", [G, P, FREE], BF16, isOutput=False)
    x4 = nc.declare_dram_parameter("x4", [G, P, P], BF16, isOutput=False)
    dbT = nc.declare_dram_parameter("dbT", [B, P], BF16, isOutput=False)
    db = nc.declare_dram_parameter("db", [P, B], BF16, isOutput=False)
    dc4 = nc.declare_dram_parameter("dc4", [P, 4], F32, isOutput=False)
    dc4T = nc.declare_dram_parameter("dc4T", [4, P], BF16, isOutput=False)
    out_v = nc.declare_dram_parameter("out_v", [B, FREE], F32, isOutput=True)

    uh_dram = nc.dram_tensor("uh_dram", [NSTR, P, FREE], BF16)
    ar_in = nc.dram_tensor("ar_in", [B, FREE], F32)
    ar_out = nc.dram_tensor("ar_out", [B, FREE], F32, addr_space="Shared")

    with tile.TileContext(nc) as tc:
        with (
            tc.tile_pool(name="persist", bufs=1) as persist,
            tc.tile_pool(name="wgp", bufs=3) as wgp,
            tc.tile_pool(name="x4p", bufs=3) as x4p,
            tc.tile_pool(name="stage", bufs=3) as stage,
            tc.tile_pool(name="scratch", bufs=4) as scratch,
            tc.tile_pool(name="agA", bufs=2) as agAp,
            tc.tile_pool(name="small", bufs=2) as small,
            tc.tile_pool(name="smallbig", bufs=2) as smallbig,
            tc.tile_pool(name="spsum", bufs=1, space="PSUM") as spsump,
        ):
            # ---- persistent tiles ----
            uh_res = persist.tile([P, RES * FREE], BF16, tag="uhres")
            dbT_s = persist.tile([B, P], BF16, tag="dbT")
            db_s = persist.tile([P, B], BF16, tag="db")
            dc4_s = persist.tile([P, 4], F32, tag="dc4")
            dc4T_s = persist.tile([4, P], BF16, tag="dc4T")
            b_log = persist.tile([4, G * U], F32, tag="blog")
            vrep = persist.tile([P, FREE], BF16, tag="vrep")
            s_sb = persist.tile([B, FREE], F32, tag="ssb")
            v32 = persist.tile([B, FREE], BF16, tag="v32")

            nc.sync.dma_start(dbT_s[:], dbT[:])
            nc.sync.dma_start(db_s[:], db[:])
            nc.sync.dma_start(dc4_s[:], dc4[:])
            nc.sync.dma_start(dc4T_s[:], dc4T[:])
            nc.vector.memset(b_log[:], 0.0)

            def ou_view(ap):
                # [p, FREE] -> [p, o, u]  (u innermost, stride 1)
                return ap.rearrange("p (o u) -> p o u", u=U)

            def uo_red_view(ap):
                # [p, FREE] -> [p, u, o] (o innermost in AP: stride U) for o-reduce
                return ap.rearrange("p (o u) -> p u o", u=U)

            def squash_and_v(s_psum, sc, is_last):
                """AllReduce s, squash -> v; build vrep (unless last) or DMA out."""
                s0 = smallbig.tile([B, FREE], F32, tag="big")
                nc.scalar.copy(s0[:], s_psum[:])
                nc.gpsimd.dma_start(ar_in[:], s0[:])
                nc.gpsimd.collective_compute(
                    "AllReduce",
                    mybir.AluOpType.add,
                    replica_groups=[list(range(NCORES))],
                    ins=[ar_in[:]],
                    outs=[ar_out[:]],
                )
                nc.gpsimd.dma_start(s_sb[:], ar_out[:])
                # a[b,u] = sum_o (sc*s)^2
                sq = smallbig.tile([B, FREE], F32, tag="big")
                nc.scalar.activation(sq[:], s_sb[:], mybir.ActivationFunctionType.Square, scale=sc)
                a = small.tile([B, U], F32, tag="a")
                nc.vector.tensor_reduce(a[:], uo_red_view(sq[:]), mybir.AxisListType.X, mybir.AluOpType.add)
                b1 = small.tile([B, U], F32, tag="b1")
                nc.vector.tensor_scalar_add(b1[:], a[:], 1.0)
                r1 = small.tile([B, U], F32, tag="r1")
                nc.vector.reciprocal(r1[:], b1[:])
                t1 = small.tile([B, U], F32, tag="t1")
                nc.vector.tensor_mul(t1[:], a[:], r1[:])
                aeps = small.tile([B, U], F32, tag="aeps")
                nc.vector.tensor_scalar_add(aeps[:], a[:], EPS)
                sg = small.tile([B, U], F32, tag="sg")
                nc.scalar.activation(sg[:], aeps[:], mybir.ActivationFunctionType.Sqrt)
                r2 = small.tile([B, U], F32, tag="r2")
                nc.vector.reciprocal(r2[:], sg[:])
                fa = small.tile([B, U], F32, tag="fa")
                nc.vector.tensor_mul(fa[:], t1[:], r2[:])
                fa2 = small.tile([B, U], F32, tag="fa2")
                nc.vector.tensor_scalar_mul(fa2[:], fa[:], float(sc))
                if is_last:
                    vf = smallbig.tile([B, FREE], F32, tag="big")
                    nc.vector.tensor_mul(
                        ou_view(vf[:]), ou_view(s_sb[:]),
                        fa2[:].unsqueeze(1).broadcast_to((B, O, U)),
                    )
                    nc.sync.dma_start(out_v[:], vf[:])
                else:
                    # v32 = v / B (bf16) used for agree; then vrep = dbT^T @ v32
                    fa3 = small.tile([B, U], F32, tag="fa3")
                    nc.vector.tensor_scalar_mul(fa3[:], fa2[:], 1.0 / B)
                    nc.vector.tensor_mul(
                        ou_view(v32[:]), ou_view(s_sb[:]),
                        fa3[:].unsqueeze(1).broadcast_to((B, O, U)),
                    )
                    with tc.tile_pool(name="vrpsum", bufs=1, space="PSUM") as vrpsum:
                        vp = vrpsum.tile([P, FREE], F32, tag="vrp")
                        for q in range(4):
                            nc.tensor.matmul(
                                vp[:, q * 512:(q + 1) * 512], dbT_s[:],
                                v32[:, q * 512:(q + 1) * 512], start=True, stop=True,
                            )
                        nc.scalar.copy(vrep[:], vp[:])

            # ================= PASS A: GEMM + s1 =================
            sp = spsump.tile([B, FREE], F32, tag="spsum")
            with tc.tile_pool(name="uhpsum", bufs=2, space="PSUM") as uhpsum:
                for gi, (kind, idx) in enumerate(_ORDER):
                    wgt = wgp.tile([P, FREE], BF16, tag="wg")
                    nc.sync.dma_start(wgt[:], wg[gi])
                    x4t = x4p.tile([P, P], BF16, tag="x4")
                    nc.sync.dma_start(x4t[:], x4[gi])
                    if kind == "res":
                        dest = uh_res[:, idx * FREE:(idx + 1) * FREE]
                    else:
                        dest_t = stage.tile([P, FREE], BF16, tag="stage")
                        dest = dest_t[:]
                    uhps = []
                    for q in range(4):
                        uhp = uhpsum.tile([P, 512], F32, tag="uhp")
                        nc.tensor.matmul(uhp[:], x4t[:], wgt[:, q * 512:(q + 1) * 512],
                                         start=True, stop=True)
                        uhps.append(uhp)
                    for q in range(4):
                        nc.scalar.copy(dest[:, q * 512:(q + 1) * 512], uhps[q][:])
                    for q in range(4):
                        nc.tensor.matmul(sp[:, q * 512:(q + 1) * 512], db_s[:],
                                         dest[:, q * 512:(q + 1) * 512],
                                         start=(gi == 0), stop=(gi == G - 1))
                    if kind == "str":
                        nc.sync.dma_start(uh_dram[idx], dest)
            squash_and_v(sp, 1.0 / U, is_last=False)

            # ============ PASS B/C: routing iterations 2,3 ============
            prod_engine = nc.gpsimd if PROD_GPSIMD else nc.vector
            for it in (2, 3):
                sp = spsump.tile([B, FREE], F32, tag="spsum")
                with (
                    tc.tile_pool(name="agpsum", bufs=2, space="PSUM") as agpsum,
                    tc.tile_pool(name="ckpsum", bufs=2, space="PSUM") as ckpsum,
                ):
                    for gi, (kind, idx) in enumerate(_ORDER):
                        if kind == "res":
                            uh = uh_res[:, idx * FREE:(idx + 1) * FREE]
                        else:
                            uht = stage.tile([P, FREE], BF16, tag="stage")
                            nc.sync.dma_start(uht[:], uh_dram[idx])
                            uh = uht[:]
                        # agree_{it-1}[c4, u] for this group's 4 channels
                        prod = scratch.tile([P, FREE], BF16, tag="ps")
                        prod_engine.tensor_mul(prod[:], uh, vrep[:])
                        agA = agAp.tile([P, U], F32, tag="agA")
                        nc.vector.tensor_reduce(agA[:], uo_red_view(prod[:]),
                                                mybir.AxisListType.X, mybir.AluOpType.add)
                        agp = agpsum.tile([4, U], F32, tag="agp")
                        nc.tensor.matmul(agp[:], dc4_s[:], agA[:], start=True, stop=True)
                        # b_log += agree ; softmax over u (local per channel)
                        bsl = b_log[:, gi * U:(gi + 1) * U]
                        nc.vector.tensor_add(bsl, bsl, agp[:])
                        e = small.tile([4, U], F32, tag="e")
                        nc.scalar.activation(e[:], bsl, mybir.ActivationFunctionType.Exp)
                        den = small.tile([4, 1], F32, tag="den")
                        nc.vector.tensor_reduce(den[:], e[:], mybir.AxisListType.X,
                                                mybir.AluOpType.add)
                        rden = small.tile([4, 1], F32, tag="rden")
                        nc.vector.reciprocal(rden[:], den[:])
                        ck = small.tile([4, U], BF16, tag="ck")
                        nc.vector.tensor_scalar_mul(ck[:], e[:], rden[:])
                        ckp = ckpsum.tile([P, U], F32, tag="ckp")
                        nc.tensor.matmul(ckp[:], dc4T_s[:], ck[:], start=True, stop=True)
                        ckrep = small.tile([P, U], BF16, tag="ckrep")
                        nc.scalar.copy(ckrep[:], ckp[:])
                        # s += ck * uh (sum over c4 via db delta-matmul)
                        sprod = scratch.tile([P, FREE], BF16, tag="ps")
                        nc.vector.tensor_mul(
                            ou_view(sprod[:]), ou_view(uh),
                            ckrep[:].unsqueeze(1).broadcast_to((P, O, U)),
                        )
                        for q in range(4):
                            nc.tensor.matmul(sp[:, q * 512:(q + 1) * 512], db_s[:],
                                             sprod[:, q * 512:(q + 1) * 512],
                                             start=(gi == 0), stop=(gi == G - 1))
                squash_and_v(sp, 1.0, is_last=(it == 3))
    nc.finalize()
    return nc


_CACHE = {}


def _get_nc():
    if "nc" not in _CACHE:
        _CACHE["nc"] = build_nc()
    return _CACHE["nc"]


def _prep_inputs(x, weight):
    """Host-side shard + layout prep. Returns per-core in_maps."""
    x = np.asarray(x, np.float32)
    weight = np.asarray(weight, np.float32)
    # W: (C, U, O, I) -> per core r: [G, 4, U, O, I] -> [G, (c4 i), (o u)]
    w8 = weight.reshape(NCORES, G, 4, U, O, I)
    wg_all = np.ascontiguousarray(
        w8.transpose(0, 1, 2, 5, 4, 3).reshape(NCORES, G, P, FREE)
    ).astype(NPBF16)
    # x: (B, I, C) -> xc (C, I, B) -> per core [G, 4, I, B] -> blockdiag [G, 128, 128]
    xc = np.ascontiguousarray(x.transpose(2, 1, 0)).reshape(NCORES, G, 4, I, B)
    x4_all = np.zeros((NCORES, G, 4, I, 4, B), np.float32)
    for c4 in range(4):
        x4_all[:, :, c4, :, c4, :] = xc[:, :, c4]
    x4_all = x4_all.reshape(NCORES, G, P, P).astype(NPBF16)

    eye_b = np.eye(B, dtype=np.float32)
    dbT_h = np.tile(eye_b, (1, 4)).astype(NPBF16)            # [B, 128]
    db_h = np.tile(eye_b, (4, 1)).astype(NPBF16)             # [128, B]
    dc4_h = np.kron(np.eye(4, dtype=np.float32), np.ones((32, 1), np.float32))  # [128,4] f32
    dc4T_h = np.kron(np.eye(4, dtype=np.float32), np.ones((1, 32), np.float32)).astype(NPBF16)

    in_maps = []
    for r in range(NCORES):
        in_maps.append({
            "wg": wg_all[r],
            "x4": x4_all[r],
            "dbT": dbT_h,
            "db": db_h,
            "dc4": dc4_h,
            "dc4T": dc4T_h,
        })
    return in_maps


def _install_ntff_shim():
    try:
        import antenv.axon_hooks  # noqa: F401
        return
    except ImportError:
        pass
    try:
        import trn_agent_boot.trn_boot as tb
        hook = tb._ntff_profile_via_ctypes("/opt/axon/libaxon_pjrt.so")
        mod = types.ModuleType("antenv.axon_hooks")
        mod.get_axon_ntff_profile_hook = lambda: hook
        sys.modules["antenv.axon_hooks"] = mod
    except Exception:
        pass


def run_on_hw(x, weight, trace=False):
    from concourse.bass_utils import run_bass_kernel_spmd

    if trace:
        _install_ntff_shim()
    nc = _get_nc()
    in_maps = _prep_inputs(x, weight)
    res = run_bass_kernel_spmd(nc, in_maps, list(range(NCORES)), trace=trace)
    v = np.asarray(res.results[0]["out_v"], np.float32)  # [B, (o,u)]
    out = np.ascontiguousarray(v.reshape(B, O, U).transpose(0, 2, 1))[..., None]
    return out, res


def kernel(x: np.ndarray, weight: np.ndarray) -> np.ndarray:
    out, _ = run_on_hw(x, weight, trace=False)
    return out


# revision 13
# speedup vs baseline: 1.7201x; 1.7201x over previous
"""CapsuleLayer dynamic-routing kernel for 8x Trainium2 (Bass/Tile).

Shapes (fixed): x (32, 32, 2048) f32, weight (2048, 64, 32, 32) f32.
B=32 batch, I=32 in_unit, C=2048 in_channel, U=64 num_unit, O=32 unit_size.

Strategy: shard C over the 8 cores (CL=256 channels each). Per core the
GEMM u_hat[b,c,u,o] = sum_i W[c,u,o,i] x[b,i,c] is computed as 64 groups
of 4 channels: K = (c4,i) = 128 with a block-diagonal x operand as the
PE-stationary weight, streaming W. u_hat tiles [(c4,b) | (o,u)] are kept
bf16, partly resident in SBUF and partly stored to DRAM. Routing
iterations 2 and 3 sweep the tiles once each, fusing agree_{k-1}
(mult+reduce, PE delta-matmul over b), the per-channel softmax (all u
local), and the weighted s_k accumulation (DVE mult, PE delta-matmul
into PSUM). s_k is AllReduced across cores (sum over the C shards);
squash runs on every core; core 0's v is the output.

Free-dim layout is (o, u) [u innermost] so the ck broadcast multiply
keeps the DVE 2x packed mode (innermost stride-1 for all operands).
"""

import os
import sys
import types

import numpy as np

_TRN_REPO = "/opt/trn_rl_repo"
if _TRN_REPO not in sys.path:
    sys.path.insert(0, _TRN_REPO)

import concourse.bacc as bacc  # noqa: E402
import concourse.bass as bass  # noqa: E402
import concourse.mybir as mybir  # noqa: E402
import concourse.tile as tile  # noqa: E402

NCORES = 8
B, I, C, U, O = 32, 32, 2048, 64, 32
CL = C // NCORES          # channels per core
G = CL // 4               # 64 groups of 4 channels
P = 128
FREE = U * O              # 2048, laid out (o, u): f = o*U + u
RES = int(os.environ.get("CAPS_RES", "20"))   # resident groups (multiple of 4)
EPS = 1e-8

F32 = mybir.dt.float32
BF16 = mybir.dt.bfloat16
NPBF16 = mybir.dt.np(BF16)

NBAT = G // 4             # 16 batches of 4 groups
RBAT = RES // 4           # resident batches (groups 0..RES-1)
NSTR = G - RES            # streamed groups
# batch processing order: interleave streamed/resident so DMA overlaps compute
_BORDER = []
_s = RBAT
_r = 0
for _k in range(NBAT):
    if _k % 2 == 0 and _s < NBAT:
        _BORDER.append(_s)
        _s += 1
    elif _r < RBAT:
        _BORDER.append(_r)
        _r += 1
    else:
        _BORDER.append(_s)
        _s += 1


def build_nc():
    nc = bacc.Bacc("TRN2", target_bir_lowering=False, debug=True)

    wg = nc.declare_dram_parameter("wg", [G, P, FREE], BF16, isOutput=False)
    x4 = nc.declare_dram_parameter("x4", [G, P, P], BF16, isOutput=False)
    dbT = nc.declare_dram_parameter("dbT", [B, P], BF16, isOutput=False)
    db = nc.declare_dram_parameter("db", [P, B], BF16, isOutput=False)
    dc4 = nc.declare_dram_parameter("dc4", [P, 4], F32, isOutput=False)
    dc4Tf = nc.declare_dram_parameter("dc4Tf", [4, P], F32, isOutput=False)
    dmask4b = nc.declare_dram_parameter("dmask4b", [P, P], BF16, isOutput=False)
    emask = nc.declare_dram_parameter("emask", [P, P], BF16, isOutput=False)
    eyemask32 = nc.declare_dram_parameter("eyemask32", [P, 32], BF16, isOutput=False)
    out_v = nc.declare_dram_parameter("out_v", [B, FREE], F32, isOutput=True)

    # streamed u_hat store, one row per streamed BATCH of 4 groups
    uh_dram = nc.dram_tensor("uh_dram", [NBAT - RBAT, P, 4 * FREE], BF16)
    ar_in = nc.dram_tensor("ar_in", [B, FREE], F32)
    ar_out = nc.dram_tensor("ar_out", [B, FREE], F32, addr_space="Shared")

    with tile.TileContext(nc) as tc:
        with (
            tc.tile_pool(name="persist", bufs=1) as persist,
            tc.tile_pool(name="stage", bufs=2) as stage,
            tc.tile_pool(name="scratch", bufs=3) as scratch,
            tc.tile_pool(name="small", bufs=2) as small,
            tc.tile_pool(name="smallbig", bufs=1) as smallbig,
        ):
            # ---- persistent tiles ----
            uh_res = persist.tile([P, RES * FREE], BF16, tag="uhres")
            dbT_s = persist.tile([B, P], BF16, tag="dbT")
            db_s = persist.tile([P, B], BF16, tag="db")
            dc4_s = persist.tile([P, 4], F32, tag="dc4")
            dc4Tf_s = persist.tile([4, P], F32, tag="dc4Tf")
            dmask_s = persist.tile([P, P], BF16, tag="dmask")
            emask_s = persist.tile([P, P], BF16, tag="emask")
            eye32_s = persist.tile([P, 32], BF16, tag="eye32")
            # routing logits, [(c4, u_half) | G] per half
            blog0 = persist.tile([P, G], F32, tag="blog0")
            blog1 = persist.tile([P, G], F32, tag="blog1")
            vdelta = persist.tile([P, 64 * P], BF16, tag="vdelta")  # [o*2+h][128]
            ckrep_all = persist.tile([P, G * U], BF16, tag="ckall")
            s_sb = persist.tile([B, FREE], F32, tag="ssb")
            v32 = persist.tile([B, FREE], BF16, tag="v32")

            nc.sync.dma_start(dbT_s[:], dbT[:])
            nc.sync.dma_start(db_s[:], db[:])
            nc.sync.dma_start(dc4_s[:], dc4[:])
            nc.sync.dma_start(dc4Tf_s[:], dc4Tf[:])
            nc.sync.dma_start(dmask_s[:], dmask4b[:])
            nc.sync.dma_start(emask_s[:], emask[:])
            nc.sync.dma_start(eye32_s[:], eyemask32[:])
            nc.vector.memset(blog0[:], 0.0)
            nc.vector.memset(blog1[:], 0.0)

            blog = [blog0, blog1]

            def ou_view(ap):
                # [p, FREE] -> [p, o, u]  (u innermost, stride 1)
                return ap.rearrange("p (o u) -> p o u", u=U)

            def uo_red_view(ap):
                # [p, FREE] -> [p, u, o] (o innermost in AP, stride U)
                return ap.rearrange("p (o u) -> p u o", u=U)

            def squash_and_v(s0, sc, is_last):
                """AllReduce s0, squash -> v; build v32 (agree operand) or DMA out."""
                nc.gpsimd.dma_start(ar_in[:], s0[:])
                nc.gpsimd.collective_compute(
                    "AllReduce",
                    mybir.AluOpType.add,
                    replica_groups=[list(range(NCORES))],
                    ins=[ar_in[:]],
                    outs=[ar_out[:]],
                )
                nc.gpsimd.dma_start(s_sb[:], ar_out[:])
                # a[b,u] = sum_o (sc*s)^2
                sq = smallbig.tile([B, FREE], F32, tag="big")
                nc.scalar.activation(sq[:], s_sb[:], mybir.ActivationFunctionType.Square, scale=sc)
                a = small.tile([B, U], F32, tag="a")
                nc.vector.tensor_reduce(a[:], uo_red_view(sq[:]), mybir.AxisListType.X, mybir.AluOpType.add)
                b1 = small.tile([B, U], F32, tag="b1")
                nc.vector.tensor_scalar_add(b1[:], a[:], 1.0)
                r1 = small.tile([B, U], F32, tag="r1")
                nc.vector.reciprocal(r1[:], b1[:])
                t1 = small.tile([B, U], F32, tag="t1")
                nc.vector.tensor_mul(t1[:], a[:], r1[:])
                aeps = small.tile([B, U], F32, tag="aeps")
                nc.vector.tensor_scalar_add(aeps[:], a[:], EPS)
                sg = small.tile([B, U], F32, tag="sg")
                nc.scalar.activation(sg[:], aeps[:], mybir.ActivationFunctionType.Sqrt)
                r2 = small.tile([B, U], F32, tag="r2")
                nc.vector.reciprocal(r2[:], sg[:])
                fa = small.tile([B, U], F32, tag="fa")
                nc.vector.tensor_mul(fa[:], t1[:], r2[:])
                fa2 = small.tile([B, U], F32, tag="fa2")
                nc.vector.tensor_scalar_mul(fa2[:], fa[:], float(sc))
                if is_last:
                    vf = smallbig.tile([B, FREE], F32, tag="big")
                    nc.vector.tensor_mul(
                        ou_view(vf[:]), ou_view(s_sb[:]),
                        fa2[:].unsqueeze(1).broadcast_to((B, O, U)),
                    )
                    nc.sync.dma_start(out_v[:], vf[:])
                else:
                    # v32 = v / B (bf16), the agree-side operand
                    fa3 = small.tile([B, U], F32, tag="fa3")
                    nc.vector.tensor_scalar_mul(fa3[:], fa2[:], 1.0 / B)
                    nc.vector.tensor_mul(
                        ou_view(v32[:]), ou_view(s_sb[:]),
                        fa3[:].unsqueeze(1).broadcast_to((B, O, U)),
                    )

            # ================= PASS A: GEMM + s1 =================
            with (
                tc.tile_pool(name="wgp", bufs=2) as wgp,
                tc.tile_pool(name="x4p", bufs=3) as x4p,
                tc.tile_pool(name="uhpsum", bufs=4, space="PSUM") as uhpsum,
                tc.tile_pool(name="spsumA", bufs=1, space="PSUM") as spsump,
            ):
                sp = spsump.tile([B, FREE], F32, tag="spsum")
                first = True
                for bo_i, bidx in enumerate(_BORDER):
                    resident = bidx < RBAT
                    if resident:
                        dest = uh_res[:, bidx * 4 * FREE:(bidx + 1) * 4 * FREE]
                    else:
                        dest_t = stage.tile([P, 4 * FREE], BF16, tag="stage")
                        dest = dest_t[:]
                    for gl in range(4):
                        gi = bidx * 4 + gl
                        wgt = wgp.tile([P, FREE], BF16, tag="wg")
                        nc.sync.dma_start(wgt[:], wg[gi])
                        x4t = x4p.tile([P, P], BF16, tag="x4")
                        nc.sync.dma_start(x4t[:], x4[gi])
                        gdest = dest[:, gl * FREE:(gl + 1) * FREE]
                        uhps = []
                        for q in range(4):
                            uhp = uhpsum.tile([P, 512], F32, tag="uhp")
                            nc.tensor.matmul(uhp[:], x4t[:], wgt[:, q * 512:(q + 1) * 512],
                                             start=True, stop=True)
                            uhps.append(uhp)
                        for q in range(4):
                            nc.scalar.copy(gdest[:, q * 512:(q + 1) * 512], uhps[q][:])
                        last = (bo_i == NBAT - 1) and (gl == 3)
                        for q in range(4):
                            nc.tensor.matmul(sp[:, q * 512:(q + 1) * 512], db_s[:],
                                             gdest[:, q * 512:(q + 1) * 512],
                                             start=first, stop=last)
                        first = False
                    if not resident:
                        nc.sync.dma_start(uh_dram[bidx - RBAT], dest)
                s0 = smallbig.tile([B, FREE], F32, tag="big")
                nc.scalar.copy(s0[:], sp[:])
            squash_and_v(s0, 1.0 / U, is_last=False)

            # ============ PASS B/C: routing iterations 2,3 ============
            for it in (2, 3):
                # --- build vdelta tiles for this iteration's v32 ---
                with tc.tile_pool(name="vdpsum", bufs=4, space="PSUM") as vdpsum:
                    for o in range(O):
                        for h in range(2):
                            vdp = vdpsum.tile([P, P], F32, tag="vdp")
                            # rhs col (c4', u'): v32[b, o*U + h*32 + u'], c4'-bcast
                            _v = v32[:]
                            rhs = bass.AP(tensor=_v.tensor, offset=_v.offset + o * U + h * 32,
                                          ap=[list(_v.ap[0]), [0, 4], [1, 32]])
                            nc.tensor.matmul(vdp[:], dbT_s[:], rhs, start=True, stop=True)
                            vd = vdelta[:, (o * 2 + h) * P:(o * 2 + h + 1) * P]
                            nc.vector.tensor_mul(vd, vdp[:], dmask_s[:])
                        # (vd is bf16 out; in0 psum f32, in1 bf16)

                # --- phase 1: agree matmuls + softmax -> ckrep_all ---
                with (
                    tc.tile_pool(name="agpsum", bufs=4, space="PSUM") as agpsum,
                    tc.tile_pool(name="ckpsum", bufs=2, space="PSUM") as ckpsum,
                    tc.tile_pool(name="denpsum", bufs=1, space="PSUM") as denpsum,
                ):
                    for pi in range(0, NBAT, 2):
                        pair = [_BORDER[pi], _BORDER[pi + 1]]
                        uhb = []
                        for bidx in pair:
                            if bidx < RBAT:
                                uhb.append(uh_res[:, bidx * 4 * FREE:(bidx + 1) * 4 * FREE])
                            else:
                                uht = stage.tile([P, 4 * FREE], BF16, tag="stage")
                                nc.sync.dma_start(uht[:], uh_dram[bidx - RBAT])
                                uhb.append(uht[:])
                        agps = []
                        for _j in range(2):
                            row = []
                            for _h in range(2):
                                agp_t = agpsum.tile([P, P], F32, tag="agp")
                                row.append(agp_t)
                            agps.append(row)
                        for o in range(O):
                            for h in range(2):
                                vd = vdelta[:, (o * 2 + h) * P:(o * 2 + h + 1) * P]
                                for j in range(2):
                                    rhs = bass.AP(tensor=uhb[j].tensor,
                                                  offset=uhb[j].offset + o * U + h * 32,
                                                  ap=[list(uhb[j].ap[0]), [FREE, 4], [1, 32]])
                                    nc.tensor.matmul(agps[j][h][:], vd, rhs,
                                                     start=(o == 0), stop=(o == O - 1))
                        for j, bidx in enumerate(pair):
                            gsl = slice(bidx * 4, bidx * 4 + 4)
                            for h in range(2):
                                # extract diag: agB[(c4,u'), g] = sum_u'' psum*emask
                                mk = scratch.tile([P, P], F32, tag="mk")
                                nc.vector.tensor_mul(mk[:], agps[j][h][:], emask_s[:])
                                agB = small.tile([P, 4], F32, tag="agB")
                                nc.vector.tensor_reduce(
                                    agB[:], mk[:].rearrange("p (g u) -> p g u", u=32),
                                    mybir.AxisListType.X, mybir.AluOpType.add)
                                bl = blog[h][:, gsl]
                                nc.vector.tensor_add(bl, bl, agB[:])
                            # softmax over u for the 4 channels of each group
                            exps = []
                            dnp = denpsum.tile([4, 4], F32, tag="den")
                            for h in range(2):
                                e = small.tile([P, 4], F32, tag="e")
                                nc.scalar.activation(e[:], blog[h][:, gsl],
                                                     mybir.ActivationFunctionType.Exp)
                                exps.append(e)
                                nc.tensor.matmul(dnp[:], dc4_s[:], e[:],
                                                 start=(h == 0), stop=(h == 1))
                            rden = small.tile([4, 4], F32, tag="rden")
                            nc.vector.reciprocal(rden[:], dnp[:])
                            rrp = denpsum.tile([P, 4], F32, tag="rrp")
                            nc.tensor.matmul(rrp[:], dc4Tf_s[:], rden[:], start=True, stop=True)
                            ckh = []
                            for h in range(2):
                                c = small.tile([P, 4], BF16, tag="ckh")
                                nc.vector.tensor_mul(c[:], exps[h][:], rrp[:])
                                ckh.append(c)
                            # ckrep[(c4,b), u] per group via ckw matmul
                            for gl in range(4):
                                gi = bidx * 4 + gl
                                ckp = ckpsum.tile([P, U], F32, tag="ckp")
                                for h in range(2):
                                    ckw = scratch.tile([P, P], BF16, tag="ckw")
                                    nc.vector.tensor_mul(
                                        ckw[:],
                                        ckh[h][:, gl:gl + 1].broadcast_to((P, P)),
                                        dmask_s[:])
                                    nc.tensor.matmul(ckp[:, h * 32:(h + 1) * 32],
                                                     ckw[:], eye32_s[:],
                                                     start=True, stop=True)
                                nc.scalar.copy(ckrep_all[:, gi * U:(gi + 1) * U], ckp[:])

                # --- phase 2: s accumulation ---
                with tc.tile_pool(name="spsumB", bufs=1, space="PSUM") as spsump2:
                    sp = spsump2.tile([B, FREE], F32, tag="spsum2")
                    first = True
                    for bo_i, bidx in enumerate(_BORDER):
                        if bidx < RBAT:
                            ub = uh_res[:, bidx * 4 * FREE:(bidx + 1) * 4 * FREE]
                        else:
                            uht = stage.tile([P, 4 * FREE], BF16, tag="stage")
                            nc.sync.dma_start(uht[:], uh_dram[bidx - RBAT])
                            ub = uht[:]
                        for gl in range(4):
                            gi = bidx * 4 + gl
                            uh = ub[:, gl * FREE:(gl + 1) * FREE]
                            sprod = scratch.tile([P, FREE], BF16, tag="ps")
                            nc.vector.tensor_mul(
                                ou_view(sprod[:]), ou_view(uh),
                                ckrep_all[:, gi * U:(gi + 1) * U]
                                .unsqueeze(1).broadcast_to((P, O, U)),
                            )
                            last = (bo_i == NBAT - 1) and (gl == 3)
                            for q in range(4):
                                nc.tensor.matmul(sp[:, q * 512:(q + 1) * 512], db_s[:],
                                                 sprod[:, q * 512:(q + 1) * 512],
                                                 start=first, stop=last)
                            first = False
                    s0 = smallbig.tile([B, FREE], F32, tag="big")
                    nc.scalar.copy(s0[:], sp[:])
                squash_and_v(s0, 1.0, is_last=(it == 3))
    nc.finalize()
    return nc



_CACHE = {}


def _get_nc():
    if "nc" not in _CACHE:
        _CACHE["nc"] = build_nc()
    return _CACHE["nc"]


def _prep_inputs(x, weight):
    """Host-side shard + layout prep. Returns per-core in_maps."""
    x = np.asarray(x, np.float32)
    weight = np.asarray(weight, np.float32)
    # W: (C, U, O, I) -> per core r: [G, 4, U, O, I] -> [G, (c4 i), (o u)]
    w8 = weight.reshape(NCORES, G, 4, U, O, I)
    wg_all = np.ascontiguousarray(
        w8.transpose(0, 1, 2, 5, 4, 3).reshape(NCORES, G, P, FREE)
    ).astype(NPBF16)
    # x: (B, I, C) -> xc (C, I, B) -> per core [G, 4, I, B] -> blockdiag [G, 128, 128]
    xc = np.ascontiguousarray(x.transpose(2, 1, 0)).reshape(NCORES, G, 4, I, B)
    x4_all = np.zeros((NCORES, G, 4, I, 4, B), np.float32)
    for c4 in range(4):
        x4_all[:, :, c4, :, c4, :] = xc[:, :, c4]
    x4_all = x4_all.reshape(NCORES, G, P, P).astype(NPBF16)

    eye_b = np.eye(B, dtype=np.float32)
    dbT_h = np.tile(eye_b, (1, 4)).astype(NPBF16)            # [B, 128]
    db_h = np.tile(eye_b, (4, 1)).astype(NPBF16)             # [128, B]
    dc4_h = np.kron(np.eye(4, dtype=np.float32), np.ones((32, 1), np.float32))  # [128,4] f32
    dc4Tf_h = np.kron(np.eye(4, dtype=np.float32), np.ones((1, 32), np.float32))  # [4,128] f32
    dmask4b_h = np.kron(np.eye(4, dtype=np.float32), np.ones((32, 32), np.float32)).astype(NPBF16)
    emask_h = np.kron(np.ones((4, 4), np.float32), np.eye(32, dtype=np.float32)).astype(NPBF16)
    eyemask32_h = np.kron(np.ones((4, 1), np.float32), np.eye(32, dtype=np.float32)).astype(NPBF16)

    in_maps = []
    for r in range(NCORES):
        in_maps.append({
            "wg": wg_all[r],
            "x4": x4_all[r],
            "dbT": dbT_h,
            "db": db_h,
            "dc4": dc4_h,
            "dc4Tf": dc4Tf_h,
            "dmask4b": dmask4b_h,
            "emask": emask_h,
            "eyemask32": eyemask32_h,
        })
    return in_maps


def _install_ntff_shim():
    try:
        import antenv.axon_hooks  # noqa: F401
        return
    except ImportError:
        pass
    try:
        import trn_agent_boot.trn_boot as tb
        hook = tb._ntff_profile_via_ctypes("/opt/axon/libaxon_pjrt.so")
        mod = types.ModuleType("antenv.axon_hooks")
        mod.get_axon_ntff_profile_hook = lambda: hook
        sys.modules["antenv.axon_hooks"] = mod
    except Exception:
        pass


def run_on_hw(x, weight, trace=False):
    from concourse.bass_utils import run_bass_kernel_spmd

    if trace:
        _install_ntff_shim()
    nc = _get_nc()
    in_maps = _prep_inputs(x, weight)
    res = run_bass_kernel_spmd(nc, in_maps, list(range(NCORES)), trace=trace)
    v = np.asarray(res.results[0]["out_v"], np.float32)  # [B, (o,u)]
    out = np.ascontiguousarray(v.reshape(B, O, U).transpose(0, 2, 1))[..., None]
    return out, res


def kernel(x: np.ndarray, weight: np.ndarray) -> np.ndarray:
    out, _ = run_on_hw(x, weight, trace=False)
    return out
